# revision 9
# baseline (speedup 1.0000x reference)
"""Trainium2 Bass kernel for nn_CliffordRollingAttention.

Strategy (head-parallel over 8 cores, 2 heads/core), v2 fused pipeline:
  - Host pre-transposes x -> xT [D, B*L] bf16, slices/folds weights per core.
  - On-device per core, per batch half:
      A: QKV projections on PE in transposed layout [d, l] (2 rotating PSUM
         banks, m-outer over 256-l subchunks), bf16 with fp32 PSUM.
         Per-row sumsq partials for RMS via ACT Square + PE ones-reduce.
      B: 64KB AllReduce of q/k sumsq partials, rsqrt -> bf16 rms rows.
      C (per 512-l chunk, fused): rms rows broadcast via gpsimd
         partition_broadcast; k-norm + qm build (PE matmul w/ host-folded
         mix matrix, DVE multiply); scores = DVE products qm*k[:, l-s]
         + PE one-hot reduce into [16, l] PSUM; ACT exp; denominator via
         PE ones-reduce + DVE reciprocal; exp rows flattened to partition 0
         (SBUF DMA) then gpsimd partition_broadcast to [128, l]; apply
         entirely in transposed layout: acc += eb_i * v^T[:, l-s] on DVE
         (v^T kept resident in SBUF, never transposed or respilled);
         normalize once by broadcast reciprocal; output projection directly
         from the transposed accumulator (PE), row-major PSUM -> bf16 out.
  - Emission interleaves batch-1 projections with batch-0 fused chunks so
    PE stays busy while DVE/GpSimd work, and vice versa.
  - Host sums the 8 partial outputs in fp32 and adds the output bias.
"""

import numpy as np
import ml_dtypes

import concourse.bass as bass
import concourse.bacc as bacc
import concourse.mybir as mybir
import concourse.tile as tile
from concourse import library_config
from concourse.bass_utils import run_bass_kernel_spmd

BF = ml_dtypes.bfloat16
FP32 = mybir.dt.float32
BF16 = mybir.dt.bfloat16

B, L, D = 2, 4096, 2048
H, DH = 16, 128
NCORES = 8
HPC = H // NCORES          # heads per core = 2
DPC = HPC * DH             # channels per core = 256
N = B * L                  # 8192 rows
EPS = 1e-6
SEQ_SHIFTS = [0, 1, -1, 3, -3, 9, -9, 26, -26, 78, -78, 232, -232, 689, -689, 2048]
CH_SHIFTS = [1, 2, 4, 8]
NS = len(SEQ_SHIFTS)       # 16
CHUNK = 512
NCHUNK = N // CHUNK        # 16
SUB = 256                  # P1 subchunk
AluOp = mybir.AluOpType
AF = mybir.ActivationFunctionType


def _wrap_runs(start, length):
    """Split output positions j in [0,length) whose source row is
    b*L + ((start + j) mod L) into maximal contiguous source runs.
    Returns list of (j_offset, src_local_start, run_len)."""
    runs = []
    j = 0
    while j < length:
        src = (start + j) % L
        run = min(length - j, L - src)
        runs.append((j, src, run))
        j += run
    return runs


def _aligned_runs(start, length):
    """_wrap_runs variant that forces every destination start (joff) even,
    so DVE writes are 4-byte aligned (odd starts cost ~4x on HW). An odd
    run start is expanded one element left (recomputing the previous run's
    last element, identical by modular arithmetic) unless the source would
    underflow, in which case a 1-element op is split off."""
    out = []
    for (joff, srcl, rl) in _wrap_runs(start, length):
        if joff % 2 == 1:
            if srcl >= 1:
                out.append((joff - 1, srcl - 1, rl + 1))
            else:
                out.append((joff, srcl, 1))
                if rl > 1:
                    out.append((joff + 1, srcl + 1, rl - 1))
        else:
            out.append((joff, srcl, rl))
    return out


def _build_program():
    nc = bacc.Bacc(num_devices=NCORES)

    handles = {
        "xT": nc.declare_dram_parameter("xT", [D, N], BF16, isOutput=False),
        "wT": nc.declare_dram_parameter("wT", [D, 6 * 128], BF16, isOutput=False),
        "bias6": nc.declare_dram_parameter("bias6", [128, 6], FP32, isOutput=False),
        "pmT": nc.declare_dram_parameter("pmT", [128, HPC * 128], BF16, isOutput=False),
        "oh": nc.declare_dram_parameter("oh", [128, NS * 16], BF16, isOutput=False),
        "ones16": nc.declare_dram_parameter("ones16", [16, 1], BF16, isOutput=False),
        "woT": nc.declare_dram_parameter("woT", [DPC, D], BF16, isOutput=False),
        "outp": nc.declare_dram_parameter("outp", [N, D], BF16, isOutput=True),
    }

    import contextlib
    with tile.TileContext(nc) as tc:
        with contextlib.ExitStack() as ctx:
            _emit_inner(ctx, tc, handles)
    nc.compile()
    return nc


def _emit_inner(ctx, tc, handles):
    nc = tc.nc
    xT = handles["xT"][:]
    wT = handles["wT"][:]
    bias6 = handles["bias6"][:]
    pmT_d = handles["pmT"][:]
    oh_d = handles["oh"][:]
    ones16_d = handles["ones16"][:]
    woT_d = handles["woT"][:]
    outp = handles["outp"][:]

    nc.gpsimd.load_library(library_config.proxy)

    # ---------------- persistent pools ----------------
    const = ctx.enter_context(tc.tile_pool(name="const", bufs=1))
    big = ctx.enter_context(tc.tile_pool(name="big", bufs=1))
    dram = ctx.enter_context(tc.tile_pool(name="dram", bufs=1, space="DRAM"))

    w_sb = const.tile([128, 16 * 768], BF16)        # 24KB
    bias_sb = const.tile([128, 6], FP32)
    pm_sb = const.tile([128, HPC * 128], BF16)
    oh_sb = const.tile([128, NS * 16], BF16)
    ones16_sb = const.tile([16, 1], BF16)
    eps_sb = const.tile([128, 1], FP32)
    wo_sb = const.tile([128, HPC * D], BF16)        # 8KB [128, dt*2048 + e]

    nc.sync.dma_start(w_sb[:].rearrange("p (k j) -> p k j", k=16),
                      wT.rearrange("(k p) j -> p k j", p=128))
    nc.sync.dma_start(bias_sb[:], bias6)
    nc.sync.dma_start(pm_sb[:], pmT_d)
    nc.sync.dma_start(oh_sb[:], oh_d)
    nc.sync.dma_start(ones16_sb[:], ones16_d)
    nc.gpsimd.memset(eps_sb[:], EPS)
    nc.sync.dma_start(wo_sb[:].rearrange("p (dt e) -> p dt e", dt=2),
                      woT_d.rearrange("(dt p) e -> p dt e", p=128))

    q_raw = big.tile([128, HPC * N], BF16)               # 32KB [p, h*N + l]
    k_sb = big.tile([128, HPC * N], BF16)                # 32KB
    v_sb = big.tile([128, HPC * N], BF16)                # 32KB

    q_view = q_raw[:].rearrange("p (h l) -> p h l", h=2)
    k_view = k_sb[:].rearrange("p (h l) -> p h l", h=2)
    v_view = v_sb[:].rearrange("p (h l) -> p h l", h=2)

    ss_dram = dram.tile([2, 2, N // 2], FP32)   # [half, q/k, l-in-half]
    ss_out = dram.tile([2, 2, N // 2], FP32)
    rms_dram = dram.tile([2, N], BF16)

    # ---------------- working pools ----------------
    p1x = ctx.enter_context(tc.tile_pool(name="p1x", bufs=2))
    sqp = ctx.enter_context(tc.tile_pool(name="sqp", bufs=2))
    fp = ctx.enter_context(tc.tile_pool(name="fp", bufs=2))
    fp1 = ctx.enter_context(tc.tile_pool(name="fp1", bufs=1))
    fpe = ctx.enter_context(tc.tile_pool(name="fpe", bufs=2))
    p1ps = ctx.enter_context(tc.tile_pool(name="p1ps", bufs=1, space="PSUM"))
    miscps = ctx.enter_context(tc.tile_pool(name="miscps", bufs=2, space="PSUM"))
    scps = ctx.enter_context(tc.tile_pool(name="scps", bufs=1, space="PSUM"))
    p7ps = ctx.enter_context(tc.tile_pool(name="p7ps", bufs=2, space="PSUM"))

    # ---------------- stage B: AllReduce + rsqrt ----------------
    def emit_rms_half(hf):
        HN = N // 2
        nc.gpsimd.collective_compute(
            "AllReduce", AluOp.add,
            replica_groups=[list(range(NCORES))],
            ins=[ss_dram[hf].opt()],
            outs=[ss_out[hf].opt()],
        )
        with tc.tile_pool(name=f"p2_{hf}", bufs=1) as p2:
            col = p2.tile([128, 64], FP32, name=f"col_{hf}")
            srt = p2.tile([128, 64], FP32, name=f"srt_{hf}")
            rinv = p2.tile([128, 64], BF16, name=f"rinv_{hf}")
            for r in range(2):
                nc.sync.dma_start(
                    col[:, r * 32:(r + 1) * 32],
                    ss_out[hf, r, :].rearrange("(t p) -> p t", p=128))
            nc.scalar.activation(srt[:], col[:], AF.Sqrt, bias=eps_sb[:],
                                 scale=1.0 / D)
            with nc.allow_low_precision(reason="bf16 rms factors, tol 2e-2"):
                nc.vector.reciprocal(rinv[:], srt[:])
            for r in range(2):
                nc.sync.dma_start(
                    rms_dram[r, hf * HN:(hf + 1) * HN]
                    .rearrange("(t p) -> p t", p=128),
                    rinv[:, r * 32:(r + 1) * 32])

    # ---------------- stage A: projections (one 256-l subchunk) ------------
    def emit_p1_sub(si):
        cs = si * SUB
        hfc, hcs = divmod(cs, N // 2)
        xt = p1x.tile([128, 16, SUB], BF16, tag="xt", name=f"xt_{si}")
        nc.sync.dma_start(xt[:],
                          xT[:, cs:cs + SUB].rearrange("(k p) l -> p k l", p=128))
        sqs = {}
        # m order: q0 q1 k0 k1 v0 v1
        for m in range(6):
            ps = p1ps.tile([128, SUB], FP32, tag=f"ps{m % 2}", name=f"ps{m}_{si}")
            for k in range(16):
                nc.tensor.matmul(
                    ps[:],
                    w_sb[:, k * 768 + 128 * m: k * 768 + 128 * (m + 1)],
                    xt[:, k, :],
                    start=(k == 0), stop=(k == 15),
                )
            kind, dt = divmod(m, 2)
            if kind == 0:    # q
                nc.scalar.activation(q_raw[:, dt * N + cs: dt * N + cs + SUB],
                                     ps[:], AF.Identity, bias=bias_sb[:, dt:dt + 1])
                sq = sqp.tile([128, SUB], BF16, tag=f"sq{dt}", name=f"sq{dt}_{si}")
                nc.scalar.activation(sq[:], ps[:], AF.Square,
                                     bias=bias_sb[:, dt:dt + 1])
                sqs[f"q{dt}"] = sq
            elif kind == 1:  # k
                nc.scalar.activation(k_sb[:, dt * N + cs: dt * N + cs + SUB],
                                     ps[:], AF.Identity, bias=bias_sb[:, 2 + dt:3 + dt])
                sq = sqp.tile([128, SUB], BF16, tag=f"sqk{dt}", name=f"sqk{dt}_{si}")
                nc.scalar.activation(sq[:], ps[:], AF.Square,
                                     bias=bias_sb[:, 2 + dt:3 + dt])
                sqs[f"k{dt}"] = sq
            else:            # v
                nc.scalar.activation(v_sb[:, dt * N + cs: dt * N + cs + SUB],
                                     ps[:], AF.Identity, bias=bias_sb[:, 4 + dt:5 + dt])
        # sumsq reduce via PE (ones column of oh), evict, ship to DRAM
        for kind, key in ((0, "q"), (1, "k")):
            ssq = miscps.tile([1, 512], FP32, tag="misc", name=f"ssq{key}_{si}")
            for dt in range(2):
                nc.tensor.matmul(ssq[:, 0:SUB], oh_sb[:, 0:1], sqs[f"{key}{dt}"][:],
                                 start=(dt == 0), stop=(dt == 1))
            ssr = sqp.tile([1, SUB], FP32, tag=f"ssr{key}", name=f"ssr{key}_{si}")
            nc.scalar.activation(ssr[:], ssq[:, 0:SUB], AF.Copy)
            nc.sync.dma_start(ss_dram[hfc, kind:kind + 1, hcs:hcs + SUB], ssr[:])

    _st = {}   # per-chunk tiles crossing pipeline stages

    def front_head(c):
        cs = c * CHUNK
        rq = fp1.tile([1, CHUNK], BF16, tag="rq", name=f"rq_{c}")
        rk = fp1.tile([1, CHUNK], BF16, tag="rk", name=f"rk_{c}")
        nc.sync.dma_start(rq[:], rms_dram[0:1, cs:cs + CHUNK])
        nc.sync.dma_start(rk[:], rms_dram[1:2, cs:cs + CHUNK])
        rqb = fp1.tile([128, CHUNK], BF16, tag="rqb", name=f"rqb_{c}")
        rkb = fp1.tile([128, CHUNK], BF16, tag="rkb", name=f"rkb_{c}")
        nc.gpsimd.partition_broadcast(rqb[:], rq[:])
        nc.gpsimd.partition_broadcast(rkb[:], rk[:])
        for dt in range(2):
            nc.vector.tensor_tensor(k_view[:, dt, cs:cs + CHUNK],
                                    k_view[:, dt, cs:cs + CHUNK], rkb[:],
                                    op=AluOp.mult)
        qmc = fp.tile([128, 2, CHUNK], BF16, tag="qot", name=f"qm_{c}")
        for h in range(HPC):
            qm_ps = miscps.tile([128, CHUNK], FP32, tag="misc", name=f"qmps_{c}_{h}")
            nc.tensor.matmul(qm_ps[:], pm_sb[:, 128 * h:128 * (h + 1)],
                             q_view[:, h, cs:cs + CHUNK], start=True, stop=True)
            nc.vector.tensor_tensor(qmc[:, h, :], qm_ps[:], rqb[:], op=AluOp.mult)
        sc = scps.tile([16, 2, CHUNK], FP32, tag="sc", name=f"sc_{c}")
        _st[('qm', c)] = qmc
        _st[('sc', c)] = sc

    def front_scores(c, i0, i1):
        cs = c * CHUNK
        b = cs // L
        w0 = cs - b * L
        bL = b * L
        qmc = _st[('qm', c)]
        sc = _st[('sc', c)]
        for i in range(i0, i1):
            s = SEQ_SHIFTS[i]
            pr = fp.tile([128, 2, CHUNK], BF16, tag="prod", name=f"pr_{c}_{i}")
            for (joff, srcl, rl) in _aligned_runs(w0 - s, CHUNK):
                for h in range(HPC):
                    nc.vector.tensor_tensor(
                        pr[:, h, joff:joff + rl],
                        qmc[:, h, joff:joff + rl],
                        k_view[:, h, bL + srcl: bL + srcl + rl],
                        op=AluOp.mult)
            for h in range(HPC):
                nc.tensor.matmul(sc[:, h, :], oh_sb[:, 16 * i:16 * (i + 1)],
                                 pr[:, h, :], start=(i == 0), stop=(i == NS - 1))

    def front_tail(c):
        sc = _st.pop(('sc', c))
        ec = fp.tile([16, 2, CHUNK], BF16, tag="expc", name=f"ec_{c}")
        lnd = fp1.tile([1, 2, CHUNK], FP32, tag="lnd", name=f"lnd_{c}")
        rcp = fp.tile([1, 2, CHUNK], BF16, tag="rrow", name=f"rcp_{c}")
        for h in range(HPC):
            nc.scalar.activation(ec[:, h, :], sc[:, h, :], AF.Exp)
        for h in range(HPC):
            dn = miscps.tile([1, CHUNK], FP32, tag="misc", name=f"dn_{c}_{h}")
            nc.tensor.matmul(dn[:], ones16_sb[:, 0:1], ec[:, h, :],
                             start=True, stop=True)
            nc.scalar.activation(lnd[:, h, :], dn[:], AF.Ln)
        with nc.allow_low_precision(reason="bf16 softmax recip, tol 2e-2"):
            nc.scalar.activation(rcp[:], lnd[:], AF.Exp, scale=-1.0)
        _st[('ec', c)] = ec
        _st[('rcp', c)] = rcp

    def back_head(c):
        rcp = _st.pop(('rcp', c))
        rbt = fp1.tile([128, 2, CHUNK], BF16, tag="rb", name=f"rb_{c}")
        nc.gpsimd.partition_broadcast(rbt[:], rcp[:])
        _st[('rb', c)] = rbt
        _st[('acc', c)] = [None, None]
        _st[('ef', c)] = {}

        def flat(w):
            ef = fpe.tile([1, 2, 2, CHUNK], BF16, tag="eflat", name=f"ef_{c}_{w}")
            nc.sync.dma_start(ef[:], _st[('ec', c)][2 * w:2 * w + 2, :, :])
            _st[('ef', c)][w] = ef
        flat(0)
        flat(1)
        _st[('flat', c)] = flat

    def back_wave(c, w):
        cs = c * CHUNK
        b = cs // L
        w0 = cs - b * L
        bL = b * L
        if w + 2 < 8:
            _st[('flat', c)](w + 2)
        ef = _st[('ef', c)].pop(w)
        accs = _st[('acc', c)]
        for j in range(2):
            i = 2 * w + j
            s = SEQ_SHIFTS[i]
            eb = fp.tile([128, 2, CHUNK], BF16, tag="eb", name=f"eb_{c}_{i}")
            nc.gpsimd.partition_broadcast(eb[:], ef[0:1, j, :, :])
            ch = 0 if i < 8 else 1
            cur = accs[ch]
            tgt = (fp1.tile([128, 2, CHUNK], BF16, tag=f"acc{ch}a", name=f"ap_{c}_{i}")
                   if cur is None else
                   fp.tile([128, 2, CHUNK], BF16, tag="prod", name=f"ap_{c}_{i}"))
            for (joff, srcl, rl) in _aligned_runs(w0 - s, CHUNK):
                for h in range(HPC):
                    nc.vector.tensor_tensor(
                        tgt[:, h, joff:joff + rl],
                        eb[:, h, joff:joff + rl],
                        v_view[:, h, bL + srcl: bL + srcl + rl],
                        op=AluOp.mult)
            if cur is None:
                accs[ch] = tgt
            else:
                # ping-pong: in-place DVE adds are ~4x slower on HW
                nxt = fp1.tile([128, 2, CHUNK], BF16,
                               tag=f"acc{ch}{'b' if (i % 8) % 2 == 1 else 'a'}",
                               name=f"acc_{c}_{i}")
                for h in range(HPC):
                    nc.vector.tensor_tensor(nxt[:, h, :], cur[:, h, :],
                                            tgt[:, h, :], op=AluOp.add)
                accs[ch] = nxt

    def back_tail(c):
        accs = _st.pop(('acc', c))
        rbt = _st.pop(('rb', c))
        _st.pop(('ec', c))
        _st.pop(('ef', c)); _st.pop(('flat', c))
        mrg = fp1.tile([128, 2, CHUNK], BF16, tag="acc0a", name=f"mrg_{c}")
        outT = fp.tile([128, 2, CHUNK], BF16, tag="qot", name=f"outT_{c}")
        for h in range(HPC):
            nc.vector.tensor_tensor(mrg[:, h, :], accs[0][:, h, :],
                                    accs[1][:, h, :], op=AluOp.add)
            nc.vector.tensor_tensor(outT[:, h, :], mrg[:, h, :],
                                    rbt[:, h, :], op=AluOp.mult)
        for t4 in range(4):
            t = c * 4 + t4
            for half in range(2):
                ost = fp.tile([128, D // 2], BF16, tag="ost", name=f"ost_{t}_{half}")
                for e2 in range(2):
                    e = half * 2 + e2
                    ops = p7ps.tile([128, 512], FP32, tag="p7", name=f"ops_{t}_{e}")
                    for dt in range(2):
                        nc.tensor.matmul(
                            ops[:],
                            outT[:, dt, 128 * t4:128 * (t4 + 1)],
                            wo_sb[:, dt * D + 512 * e: dt * D + 512 * (e + 1)],
                            start=(dt == 0), stop=(dt == 1))
                    nc.scalar.activation(ost[:, 512 * e2:512 * (e2 + 1)], ops[:],
                                         AF.Copy)
                nc.scalar.dma_start(
                    outp[128 * t:128 * (t + 1), half * (D // 2):(half + 1) * (D // 2)],
                    ost[:])

    def emit_front(c):
        front_head(c)
        front_scores(c, 0, NS)
        front_tail(c)

    def emit_mid(c):
        """back(c) interleaved wave-by-wave with front(c+1)."""
        back_head(c)
        front_head(c + 1)
        for w in range(8):
            back_wave(c, w)
            front_scores(c + 1, 2 * w, 2 * w + 2)
        back_tail(c)
        front_tail(c + 1)

    def emit_back(c):
        back_head(c)
        for w in range(8):
            back_wave(c, w)
        back_tail(c)

    # ---------------- schedule ----------------
    SPC = CHUNK // SUB  # subchunks per chunk = 2

    def p1c(c):
        for s in range(SPC):
            emit_p1_sub(c * SPC + s)

    for c in range(8):
        p1c(c)
    emit_rms_half(0)
    p1c(8)
    p1c(9)
    emit_front(0)
    for c in range(7):
        emit_mid(c)
        if c < 6:
            p1c(10 + c)
    emit_rms_half(1)
    for c in range(7, 15):
        emit_mid(c)
    emit_back(15)


_PROG = None
def _get_program():
    global _PROG
    if _PROG is None:
        _PROG = _build_program()
    return _PROG


def _host_prep(inputs):
    wq = np.asarray(inputs['wq'], np.float32)
    wk = np.asarray(inputs['wk'], np.float32)
    wv = np.asarray(inputs['wv'], np.float32)
    bq = np.asarray(inputs['bq'], np.float32)
    bk = np.asarray(inputs['bk'], np.float32)
    bv = np.asarray(inputs['bv'], np.float32)
    qnw = np.asarray(inputs['q_norm_w'], np.float32)
    knw = np.asarray(inputs['k_norm_w'], np.float32)
    mix = np.asarray(inputs['score_mix_w'], np.float32)[0]
    wo = np.asarray(inputs['wo'], np.float32)

    x = np.asarray(inputs['x'], np.float32)
    xT = np.ascontiguousarray(x.reshape(N, D).T).astype(BF)
    scale = DH ** -0.5

    oh = np.zeros((128, NS * 16), np.float32)
    for i in range(NS):
        oh[:, 16 * i + i] = 1.0
    oh = oh.astype(BF)
    ones16 = np.ones((16, 1), np.float32).astype(BF)

    in_maps = []
    for c in range(NCORES):
        cs = c * DPC
        sl = slice(cs, cs + DPC)
        wTc = np.concatenate([wq[sl].T, wk[sl].T, wv[sl].T], axis=1)  # [2048, 768]
        bias = np.stack([bq[cs:cs + 128], bq[cs + 128:cs + 256],
                         bk[cs:cs + 128], bk[cs + 128:cs + 256],
                         bv[cs:cs + 128], bv[cs + 128:cs + 256]], axis=1)
        pmT = np.zeros((128, HPC * 128), np.float32)
        for h in range(HPC):
            gh = c * HPC + h
            Pm = np.zeros((DH, DH), np.float32)
            for n, ch in enumerate([0] + CH_SHIFTS):
                for dd in range(DH):
                    dp = (dd - ch) % DH
                    Pm[dd, dp] += mix[n] * qnw[gh * DH + dp]
            Pm *= scale * knw[gh * DH:(gh + 1) * DH][:, None]
            pmT[:, 128 * h:128 * (h + 1)] = Pm.T
        woTc = np.ascontiguousarray(wo[:, sl].T)  # [256, 2048]
        in_maps.append({
            "xT": xT,
            "wT": wTc.astype(BF),
            "bias6": np.ascontiguousarray(bias),
            "pmT": pmT.astype(BF),
            "oh": oh,
            "ones16": ones16,
            "woT": woTc.astype(BF),
        })
    return in_maps


LAST_RESULT = None


def kernel(**inputs):
    global LAST_RESULT
    import os
    in_maps = _host_prep(inputs)
    nc = _get_program()
    trace = bool(os.environ.get("CRA_TRACE"))
    res = run_bass_kernel_spmd(nc, in_maps, list(range(NCORES)), trace=trace)
    LAST_RESULT = res
    acc = np.zeros((N, D), np.float32)
    for r in res.results:
        acc += np.asarray(r["outp"], np.float32)
    acc += np.asarray(inputs['bo'], np.float32)
    return acc.reshape(B, L, D)
